# revision 1
# baseline (speedup 1.0000x reference)
"""Causal self-attention on 8 TRN2 NeuronCores.

Problem: x[4, 2048, 1024], w_qkv[3072, 1024], w_proj[1024, 1024],
16 heads x 64 dims, causal softmax attention, output [4, 2048, 1024].

Sharding: core c handles (batch b = c//2, head-group hg = c%2).
Each head-group = 8 heads = 512 channels. Tensor-parallel over heads:
each core computes a *partial* projection output [2048, 1024]; the host
sums the two head-group partials per batch (the "all-reduce" of TP).

Per-core dataflow (all matmuls fp32r = full-rate PE):
  Phase A:  QT = Wq @ X^T   [512, 2048]   (head dims on partitions)
            KT = Wk @ X^T   [512, 2048]
            V  = X @ Wv^T   [2048, 512]   (+ ones column per head)
  Phase B (per 512-query block QI, per head h):
            ST_j = K_h^T Q_h  -> PSUM [128 keys, 512 queries]
            diagonal tiles:  ST += (-1e5 * I) @ staircase   (causal mask)
            PT_j = exp(0.125 * ST_j)            (ACT, masked lanes -> 0)
            YT  += [V_h | 1]^T @ PT_j           (accumulate over key tiles)
            row 64 of YT = softmax denominators (free-dim indexed)
  Normalize: r = 1/denoms; R = E_pc^T @ r broadcasts r over the 64
            partition rows of each head; YT *= R.
  Proj:     out = YT^T-contracted with w_proj slice -> [2048, 1024] partial.
"""

import numpy as np
from contextlib import ExitStack

import concourse.bass as bass
import concourse.tile as tile
from concourse import bacc, mybir
from concourse.bass_utils import run_bass_kernel_spmd

B, T, C, H, D = 4, 2048, 1024, 16, 64
HG = 2                 # head groups (tensor-parallel ways)
HPG = H // HG          # 8 heads per group
CG = HPG * D           # 512 channels per group
P = 128
NQI = T // 512         # 4 query blocks
NJT = T // P           # 16 key tiles
NEG = -1.0e5           # causal mask additive constant (exp(0.125*NEG) == 0)
F32 = mybir.dt.float32
F32R = mybir.dt.float32r

_CACHE = {}


def _build_core_program():
    nc = bacc.Bacc("TRN2", target_bir_lowering=False, debug=False, num_devices=8)
    xt = nc.dram_tensor("xt", [C, T], F32R, kind="ExternalInput").ap()
    wqkvt = nc.dram_tensor("wqkvt", [C, 3 * CG], F32R, kind="ExternalInput").ap()
    wpt = nc.dram_tensor("wpt", [CG, C], F32R, kind="ExternalInput").ap()
    out = nc.dram_tensor("out", [T, C], F32, kind="ExternalOutput").ap()

    with tile.TileContext(nc) as tc:
        with ExitStack() as ctx:
            _attention(ctx, tc, xt, wqkvt, wpt, out)
    nc.compile()
    return nc


def _attention(ctx, tc, xt, wqkvt, wpt, out):
    nc = tc.nc

    persist = ctx.enter_context(tc.tile_pool(name="persist", bufs=1))
    qt = persist.tile([P, 4, T], F32R, tag="qt")       # QT[c*128+p, i] at [p, c, i]
    kt = persist.tile([P, 4, T], F32R, tag="kt")
    v = persist.tile([P, NJT, HPG * 65], F32R, tag="v")  # [V_h | 1] per key tile
    ytu = persist.tile([P, 4, T], F32R, tag="ytu")     # normalized YT

    consts = ctx.enter_context(tc.tile_pool(name="consts", bufs=1))
    cstage_ctx = ExitStack()
    stage = cstage_ctx.enter_context(tc.tile_pool(name="cstage", bufs=2))
    # staircase "not-valid" masks for the 4 diagonal offsets.
    # affine_select can't write f32r directly (verifier wants a rounding
    # producer), so build each const in f32 and DVE-copy into f32r.
    notvalid = []
    for o in range(4):
        ms = stage.tile([P, 512], F32, tag="cst", name=f"nvs{o}")
        nc.gpsimd.memset(ms, 0.0)
        # keep 0 where (y - p - 128*o) >= 0 (valid), else fill 1 (masked)
        nc.gpsimd.affine_select(
            out=ms, in_=ms, compare_op=mybir.AluOpType.is_ge, fill=1.0,
            base=-128 * o, pattern=[[1, 512]], channel_multiplier=-1,
        )
        m = consts.tile([P, 512], F32R, tag=f"nv{o}", name=f"nv{o}")
        nc.vector.tensor_copy(m, ms)
        notvalid.append(m)
    ns = stage.tile([P, P], F32, tag="cst", name="negIs")
    nc.gpsimd.memset(ns, 0.0)
    nc.gpsimd.affine_select(
        out=ns, in_=ns, compare_op=mybir.AluOpType.not_equal, fill=NEG,
        base=0, pattern=[[-1, P]], channel_multiplier=1,
    )
    neg_i = consts.tile([P, P], F32R, tag="negI")
    nc.vector.tensor_copy(neg_i, ns)
    # E matrices: e8[pc][h, c] = 1 iff chunk-pc channel c belongs to head h
    e8 = []
    for pc in range(4):
        es = stage.tile([8, P], F32, tag="cste", name=f"e8s{pc}")
        nc.gpsimd.memset(es, 0.0)
        # row h, cols: head index of col c is 2*pc + c//64 -> fill 1 on match:
        # (h - 2*pc - c//64) == 0.  pattern [[-1,2],[0,64]] over free [2, 64]
        e2d = es.rearrange("h (a b) -> h a b", a=2)
        nc.gpsimd.affine_select(
            out=e2d, in_=e2d, compare_op=mybir.AluOpType.not_equal, fill=1.0,
            base=-2 * pc, pattern=[[-1, 2], [0, 64]], channel_multiplier=1,
        )
        e = consts.tile([8, P], F32R, tag=f"e8_{pc}", name=f"e8_{pc}")
        nc.vector.tensor_copy(e, es)
        e8.append(e)
    cstage_ctx.close()   # release staging SBUF before phase A pools
    # ones columns of V (col 64 of each 65-wide head slot)
    v_h = v.rearrange("p j (h e) -> p j h e", e=65)
    nc.gpsimd.memset(v_h[:, :, :, 64:65].bitcast(F32), 1.0)

    # ---------------- Phase A: QT, KT, V projections ----------------
    # Two passes over halves of the contraction dim C to bound SBUF.
    with ExitStack() as actx:
        a_x = actx.enter_context(tc.tile_pool(name="phaseA_x", bufs=5))
        a_w = actx.enter_context(tc.tile_pool(name="phaseA_w", bufs=4))
        a_psum = actx.enter_context(
            tc.tile_pool(name="phaseA_ps", bufs=3, space="PSUM")
        )

        for kp in range(2):
            xth = []
            wh = []
            for cc in range(4):
                g = (kp * 4 + cc) * P
                xc = a_x.tile([P, T], F32R, tag="xt", name=f"x{kp}{cc}")
                for ib in range(4):
                    nc.sync.dma_start(
                        xc[:, ib * 512:(ib + 1) * 512],
                        xt[g:g + P, ib * 512:(ib + 1) * 512],
                    )
                wc = a_w.tile([P, 3 * CG], F32R, tag="w", name=f"w{kp}{cc}")
                for part in range(3):
                    nc.sync.dma_start(
                        wc[:, part * CG:(part + 1) * CG],
                        wqkvt[g:g + P, part * CG:(part + 1) * CG],
                    )
                xth.append(xc)
                wh.append(wc)

            # QT rows m<4 from Wq cols, KT rows m>=4 from Wk cols
            for m in range(8):
                dst, mc = (qt, m) if m < 4 else (kt, m - 4)
                wcol = (0 if m < 4 else CG) + mc * P
                for ib in range(4):
                    ps = a_psum.tile([P, 512], F32, tag="aps")
                    for cc in range(4):
                        nc.tensor.matmul(
                            ps, wh[cc][:, wcol:wcol + P],
                            xth[cc][:, ib * 512:(ib + 1) * 512],
                            start=(cc == 0), stop=(cc == 3),
                        )
                    dslice = dst[:, mc, ib * 512:(ib + 1) * 512]
                    if kp == 0:
                        nc.scalar.copy(dslice, ps)
                    else:
                        nc.vector.tensor_add(out=dslice, in0=ps, in1=dslice)
            # V tiles
            for it in range(NJT):
                ps = a_psum.tile([P, 512], F32, tag="aps")
                for cc in range(4):
                    nc.tensor.matmul(
                        ps, xth[cc][:, it * P:(it + 1) * P],
                        wh[cc][:, 2 * CG:3 * CG],
                        start=(cc == 0), stop=(cc == 3),
                    )
                vdst = v_h[:, it, :, 0:64]
                psv = ps.rearrange("p (h e) -> p h e", e=64)
                if kp == 0:
                    nc.scalar.copy(vdst, psv)
                else:
                    nc.vector.tensor_add(out=vdst, in0=psv, in1=vdst)

    # w_proj slice (loaded once; used in phase B) — own pool, after phase A
    # pools released their SBUF
    wpt_pool = ctx.enter_context(tc.tile_pool(name="wpt", bufs=1))
    wpt_sb = wpt_pool.tile([P, 4, C], F32R, tag="wpt")
    for pc in range(4):
        for nb in range(2):
            nc.sync.dma_start(
                wpt_sb[:, pc, nb * 512:(nb + 1) * 512],
                wpt[pc * P:(pc + 1) * P, nb * 512:(nb + 1) * 512],
            )

    # ---------------- Phase B: attention ----------------
    st_ps = ctx.enter_context(tc.tile_pool(name="st_ps", bufs=2, space="PSUM"))
    yt_ps_pool = ctx.enter_context(tc.tile_pool(name="yt_ps", bufs=1, space="PSUM"))
    misc_ps = ctx.enter_context(tc.tile_pool(name="misc_ps", bufs=1, space="PSUM"))
    pt_pool = ctx.enter_context(tc.tile_pool(name="pt", bufs=3))
    sc_pool = ctx.enter_context(tc.tile_pool(name="sc", bufs=1))
    d_pool = ctx.enter_context(tc.tile_pool(name="d", bufs=2))
    r_pool = ctx.enter_context(tc.tile_pool(name="r", bufs=1))
    o_pool = ctx.enter_context(tc.tile_pool(name="o", bufs=2))

    for qi in range(NQI):
        njt = 4 * qi + 4          # key tiles in causal range for this block
        d_q = d_pool.tile([8, 512], F32, tag="dq")   # denoms, row = head
        scratch = {}              # per-head unnormalized [Y_h; denom]
        for hp in range(4):       # head pairs -> partition rows 0-63 / 64-127
            yt_tiles = [
                yt_ps_pool.tile([65, 512], F32, tag=f"yt{s}", name=f"yt{s}")
                for s in range(2)
            ]
            for j in range(njt):
                for s in range(2):
                    h = 2 * hp + s
                    r0 = s * 64
                    diag = j >= 4 * qi
                    st = st_ps.tile([P, 512], F32, tag=f"st{s}")
                    nc.tensor.matmul(
                        st, kt[r0:r0 + 64, hp, j * P:(j + 1) * P],
                        qt[r0:r0 + 64, hp, qi * 512:(qi + 1) * 512],
                        start=True, stop=not diag,
                    )
                    if diag:
                        nc.tensor.matmul(
                            st, neg_i, notvalid[j - 4 * qi],
                            start=False, stop=True,
                        )
                    pt = pt_pool.tile([P, 512], F32R, tag=f"pt{s}")
                    nc.scalar.activation(
                        pt, st, mybir.ActivationFunctionType.Exp, scale=0.125
                    )
                    nc.tensor.matmul(
                        yt_tiles[s], v[:, j, h * 65:(h + 1) * 65], pt,
                        start=(j == 0), stop=(j == njt - 1),
                    )
            for s in range(2):
                h = 2 * hp + s
                # unnormalized [Y_h; denom] -> SBUF scratch, then DMA the
                # denom row into d_q (DMA writes any partition; engines
                # can only address 32-aligned partition bases)
                sc = sc_pool.tile([65, 512], F32R, tag=f"sc{h}", name=f"sc{h}")
                nc.scalar.copy(sc, yt_tiles[s][:, :])
                nc.sync.dma_start(d_q[h:h + 1, :], sc[64:65, :].bitcast(F32))
                scratch[h] = sc

        # normalize this query block: R = e8^T @ (1/denoms)
        r_q = r_pool.tile([8, 512], F32, tag="rq")
        nc.vector.reciprocal(r_q, d_q)
        r_qr = r_pool.tile([8, 512], F32R, tag="rqr")
        nc.vector.tensor_copy(r_qr, r_q)
        for pc in range(4):
            rps = misc_ps.tile([P, 512], F32, tag="rps")
            nc.tensor.matmul(rps, e8[pc], r_qr, start=True, stop=True)
            for s in range(2):
                h = 2 * pc + s
                nc.vector.tensor_mul(
                    out=ytu[s * 64:s * 64 + 64, pc, qi * 512:(qi + 1) * 512],
                    in0=scratch[h][0:64, :],
                    in1=rps[s * 64:s * 64 + 64, :],
                )

        # projection for this query block's 4 row tiles
        for it in range(4 * qi, 4 * qi + 4):
            for nb in range(2):
                ops = misc_ps.tile([P, 512], F32, tag="proj")
                for pc in range(4):
                    nc.tensor.matmul(
                        ops, ytu[:, pc, it * P:(it + 1) * P],
                        wpt_sb[:, pc, nb * 512:(nb + 1) * 512],
                        start=(pc == 0), stop=(pc == 3),
                    )
                osb = o_pool.tile([P, 512], F32, tag="osb")
                nc.scalar.copy(osb, ops)
                nc.sync.dma_start(
                    out[it * P:(it + 1) * P, nb * 512:(nb + 1) * 512], osb
                )


def _prep_inputs(x, w_qkv, w_proj):
    """Build the 8 per-core input maps (host-side sharding + transposes)."""
    xts = [np.ascontiguousarray(x[b].T) for b in range(B)]
    wqkvts, wpts = [], []
    for hg in range(HG):
        s = hg * CG
        wq = w_qkv[s:s + CG]
        wk = w_qkv[C + s:C + s + CG]
        wv = w_qkv[2 * C + s:2 * C + s + CG]
        wqkvts.append(np.ascontiguousarray(np.concatenate([wq, wk, wv], 0).T))
        wpts.append(np.ascontiguousarray(w_proj[:, s:s + CG].T))
    in_maps = []
    for c in range(8):
        b, hg = c // 2, c % 2
        in_maps.append({"xt": xts[b], "wqkvt": wqkvts[hg], "wpt": wpts[hg]})
    return in_maps


def kernel(x, w_qkv, w_proj):
    x = np.asarray(x, dtype=np.float32)
    w_qkv = np.asarray(w_qkv, dtype=np.float32)
    w_proj = np.asarray(w_proj, dtype=np.float32)

    if "nc" not in _CACHE:
        _CACHE["nc"] = _build_core_program()
    nc = _CACHE["nc"]

    in_maps = _prep_inputs(x, w_qkv, w_proj)
    res = run_bass_kernel_spmd(nc, in_maps, core_ids=list(range(8)))
    outs = [r["out"] for r in res.results]
    full = np.empty((B, T, C), dtype=np.float32)
    for b in range(B):
        full[b] = outs[2 * b] + outs[2 * b + 1]
    return full



# revision 3
# speedup vs baseline: 1.4436x; 1.4436x over previous
"""Causal self-attention on 8 TRN2 NeuronCores — bf16 tensor-parallel version.

Problem: x[4, 2048, 1024], w_qkv[3072, 1024], w_proj[1024, 1024],
16 heads x 64 dims, causal softmax attention, output [4, 2048, 1024].

Sharding: core c handles (batch b = c//2, head-group hg = c%2).
Each head-group = 8 heads = 512 channels. Tensor-parallel over heads:
each core computes a *partial* projection output [2048, 1024] in bf16;
the host sums the two head-group partials per batch in fp32.

All matmuls are bf16 (rel-err budget 2e-2; bf16 keeps us ~1e-3):
  Phase A:  QT = Wq @ X^T   [512, 2048]   (head-pair channels on partitions)
            KT = Wk @ X^T   [512, 2048]
            V  = X @ Wv^T   [2048, 512]   (+ ones column per head)
            emitted as 8-matmul groups; KT/QT(hp>0) and V(j>3) groups are
            drained one-per-j inside the attention loops so PE fills the
            ACT-paced softmax era and ACT starts exp'ing early.
  Attention per (head-pair hp, 512-query block qi), per 128-key tile j:
            ST pair: two K=64 row-group-concurrent matmuls -> one
              [128, 1024] 2-bank PSUM tile (head s at cols 512s..)
            diagonal tiles truncated to valid queries (n0 = 128*(j-4qi));
              the leading 128x128 triangle masked by accumulating
              (-1e5*I) @ tri  (one extra N=128 matmul per head)
            PT pair = exp(0.125 * ST) in ONE activation over both heads
            YT_s += [V_h | 1]^T @ PT_s  accumulated over j  ([65, 512] PSUM;
              row 64 = softmax denominators)
  Normalize: sc_s = copy(YT_s) (DVE, f32); r = approx-recip of row 64;
            rps = e8c^T @ r broadcasts r over the 64 dims of each head;
            ytu = sc * rps  (bf16).  The rps matmul + muls are deferred
            into the next iteration's j-loop to keep the PE FIFO moving.
  Proj:     out[it*128.., nb*512..] partial = sum_pc ytu_pc^T-contracted
            with w_proj slice; bf16 out via DMA. Emitted as deferred
            groups after hp==3 so they overlap the next query block.
"""

import numpy as np
from collections import deque
from contextlib import ExitStack

import ml_dtypes

import concourse.bass as bass
import concourse.tile as tile
from concourse import bacc, mybir
from concourse.bass_utils import run_bass_kernel_spmd

B, T, C, H, D = 4, 2048, 1024, 16, 64
HG = 2                 # head groups (tensor-parallel ways)
CG = 512               # channels per head group
P = 128
NQI = T // 512         # 4 query blocks
NJT = T // P           # 16 key tiles
NEG = -1.0e5           # causal mask additive constant (exp(0.125*NEG) == 0)
F32 = mybir.dt.float32
BF16 = mybir.dt.bfloat16
BF16_NP = ml_dtypes.bfloat16

_CACHE = {}


def _build_core_program():
    nc = bacc.Bacc("TRN2", target_bir_lowering=False, debug=False, num_devices=8)
    xt = nc.dram_tensor("xt", [C, T], BF16, kind="ExternalInput").ap()
    wqkvt = nc.dram_tensor("wqkvt", [C, 3 * CG], BF16, kind="ExternalInput").ap()
    wpt = nc.dram_tensor("wpt", [CG, C], BF16, kind="ExternalInput").ap()
    out = nc.dram_tensor("out", [T, C], BF16, kind="ExternalOutput").ap()

    with tile.TileContext(nc) as tc:
        with ExitStack() as ctx:
            _attention(ctx, tc, xt, wqkvt, wpt, out)
    nc.compile()
    return nc


def _attention(ctx, tc, xt, wqkvt, wpt, out):
    nc = tc.nc

    # ---------------- persistent SBUF ----------------
    persist = ctx.enter_context(tc.tile_pool(name="persist", bufs=1))
    qt = persist.tile([P, 4, T], BF16, tag="qt")       # QT[hp*128+p, t] at [p, hp, t]
    kt = persist.tile([P, 4, T], BF16, tag="kt")
    v = persist.tile([P, NJT, 8 * 65], BF16, tag="v")  # [V_h | 1] per key tile
    ytu = persist.tile([P, 4, T], BF16, tag="ytu")     # normalized YT (bf16)
    wpt_sb = persist.tile([P, 4, C], BF16, tag="wpt")
    r_f = persist.tile([33, 512], F32, tag="rf")       # 1/denom rows {0, 32}
    r_b = persist.tile([33, 512], BF16, tag="rb")      # bf16 copy (rows 1-31 = 0)

    # ---------------- constants ----------------
    consts = ctx.enter_context(tc.tile_pool(name="consts", bufs=1))
    with ExitStack() as cstage:
        stage = cstage.enter_context(tc.tile_pool(name="cstage", bufs=2))
        ns = stage.tile([P, P], F32, tag="cst", name="negIs")
        nc.gpsimd.memset(ns, 0.0)
        # keep 0 where (q - p) != 0, fill NEG on the diagonal -> NEG * I
        nc.gpsimd.affine_select(
            out=ns, in_=ns, compare_op=mybir.AluOpType.not_equal, fill=NEG,
            base=0, pattern=[[-1, P]], channel_multiplier=1,
        )
        neg_i = consts.tile([P, P], BF16, tag="negI")
        nc.vector.tensor_copy(neg_i, ns)

        ts_ = stage.tile([P, P], F32, tag="cst", name="tris")
        nc.gpsimd.memset(ts_, 0.0)
        # keep 0 where (q - p) >= 0 (valid), fill 1 where q < p (masked)
        nc.gpsimd.affine_select(
            out=ts_, in_=ts_, compare_op=mybir.AluOpType.is_ge, fill=1.0,
            base=0, pattern=[[1, P]], channel_multiplier=-1,
        )
        tri = consts.tile([P, P], BF16, tag="tri")
        nc.vector.tensor_copy(tri, ts_)

        es = stage.tile([33, P], F32, tag="cst2", name="e8s")
        nc.gpsimd.memset(es, 0.0)
        nc.gpsimd.memset(es[0:1, 0:64], 1.0)
        nc.gpsimd.memset(es[32:33, 64:128], 1.0)
        e8c = consts.tile([33, P], BF16, tag="e8c")
        nc.vector.tensor_copy(e8c, es)

    nc.gpsimd.memset(r_b, 0.0)       # rows 1-31 must be 0 (and non-NaN)
    v4 = v.rearrange("p j (h f) -> p j h f", f=65)
    nc.gpsimd.memset(v4[:, :, :, 64:65], 1.0)   # ones column of each head

    # ---------------- input DMA ----------------
    xw = ctx.enter_context(tc.tile_pool(name="xw", bufs=8))
    xs, ws = [], []
    for cc in range(8):
        xc = xw.tile([P, T], BF16, tag="x", name=f"x{cc}")
        for h in range(2):
            nc.sync.dma_start(
                xc[:, h * 1024:(h + 1) * 1024],
                xt[cc * P:(cc + 1) * P, h * 1024:(h + 1) * 1024],
            )
        wc = xw.tile([P, 3 * CG], BF16, tag="w", name=f"w{cc}")
        for h in range(2):
            nc.sync.dma_start(
                wc[:, h * 768:(h + 1) * 768],
                wqkvt[cc * P:(cc + 1) * P, h * 768:(h + 1) * 768],
            )
        xs.append(xc)
        ws.append(wc)
    for pc in range(4):
        nc.sync.dma_start(wpt_sb[:, pc, :], wpt[pc * P:(pc + 1) * P, :])

    # ---------------- phase A group emitters ----------------
    def kq_group(pool, tag, dst, hp, ib, wbase):
        def emit():
            ps = pool.tile([P, 512], F32, tag=tag)
            for cc in range(8):
                nc.tensor.matmul(
                    ps, ws[cc][:, wbase + hp * P: wbase + hp * P + P],
                    xs[cc][:, ib * 512:(ib + 1) * 512],
                    start=(cc == 0), stop=(cc == 7),
                )
            nc.vector.tensor_copy(dst[:, hp, ib * 512:(ib + 1) * 512], ps)
        return emit

    def v_group(pool, tag, it):
        def emit():
            ps = pool.tile([P, 512], F32, tag=tag)
            for cc in range(8):
                nc.tensor.matmul(
                    ps, xs[cc][:, it * P:(it + 1) * P],
                    ws[cc][:, 2 * CG:3 * CG],
                    start=(cc == 0), stop=(cc == 7),
                )
            psv = ps.rearrange("p (h e) -> p h e", e=64)
            nc.vector.tensor_copy(v4[:, it, :, 0:64], psv)
        return emit

    # ---------------- upfront phase A (own 2-bank PSUM pool) ----------------
    with ExitStack() as upctx:
        up_ps = upctx.enter_context(
            tc.tile_pool(name="up_ps", bufs=2, space="PSUM")
        )
        for ib in range(4):
            kq_group(up_ps, "up", kt, 0, ib, CG)()
            kq_group(up_ps, "up", qt, 0, ib, 0)()
        for it in range(4):
            v_group(up_ps, "up", it)()

    # ---------------- attention-era pools (8 PSUM banks exactly) ----------
    st_ps = ctx.enter_context(tc.tile_pool(name="st_ps", bufs=2, space="PSUM"))
    yt_ps = ctx.enter_context(tc.tile_pool(name="yt_ps", bufs=1, space="PSUM"))
    rps_ps = ctx.enter_context(tc.tile_pool(name="rps_ps", bufs=1, space="PSUM"))
    aux_ps = ctx.enter_context(tc.tile_pool(name="aux_ps", bufs=1, space="PSUM"))
    pt_pool = ctx.enter_context(tc.tile_pool(name="pt", bufs=3))
    sc_pool = ctx.enter_context(tc.tile_pool(name="sc", bufs=2))
    osb_pool = ctx.enter_context(tc.tile_pool(name="osb", bufs=2))

    # remaining phase A work, drained one group per j inside attention
    pending = deque()
    for it in range(4, NJT):
        pending.append(v_group(aux_ps, "aux", it))
    for hp in range(1, 4):
        for ib in range(4):
            pending.append(kq_group(aux_ps, "aux", kt, hp, ib, CG))
            pending.append(kq_group(aux_ps, "aux", qt, hp, ib, 0))

    # ---------------- attention ----------------
    deferred = deque()    # prev iteration's rps matmul + ytu muls, proj groups
    for hp in range(4):
        for qi in range(NQI):
            njt = 4 * qi + 4
            yts = [yt_ps.tile([65, 512], F32, tag=f"yt{s}", name=f"yt{s}")
                   for s in range(2)]
            yt_pend = None
            for j in range(njt):
                o = j - 4 * qi          # >= 0 on diagonal tiles
                n0 = 128 * o if o >= 0 else 0
                stp = st_ps.tile([P, 1024], F32, tag="st")
                for s in range(2):
                    r0 = 64 * s
                    nc.tensor.matmul(
                        stp[:, s * 512 + n0: (s + 1) * 512],
                        kt[r0:r0 + 64, hp, j * P:(j + 1) * P],
                        qt[r0:r0 + 64, hp, qi * 512 + n0: (qi + 1) * 512],
                        start=True, stop=(o < 0),
                    )
                    if o >= 0:
                        nc.tensor.matmul(
                            stp[:, s * 512 + n0: s * 512 + n0 + P],
                            neg_i, tri, start=False, stop=True,
                        )
                ptp = pt_pool.tile([P, 1024], BF16, tag="pt")
                stv = stp.rearrange("p (s q) -> p s q", s=2)[:, :, n0:512]
                ptv = ptp.rearrange("p (s q) -> p s q", s=2)[:, :, n0:512]
                nc.scalar.activation(
                    ptv, stv, mybir.ActivationFunctionType.Exp, scale=0.125
                )
                if deferred:
                    deferred.popleft()()
                if pending:
                    pending.popleft()()
                if yt_pend is not None:
                    yt_pend()

                def mk_yt(j=j, n0=n0, ptp=ptp, hp=hp, last=(j == njt - 1)):
                    def e():
                        for s in range(2):
                            h = 2 * hp + s
                            nc.tensor.matmul(
                                yts[s][:, n0:512],
                                v[:, j, h * 65:(h + 1) * 65],
                                ptp[:, s * 512 + n0:(s + 1) * 512],
                                start=(j == 0), stop=last,
                            )
                    return e
                yt_pend = mk_yt()
            yt_pend()
            while deferred:
                deferred.popleft()()

            # normalize: copy out YT (frees its banks), recip the denom rows
            scs = []
            for s in range(2):
                sc = sc_pool.tile([65, 512], F32, tag=f"sc{s}", name=f"sc{s}")
                nc.vector.tensor_copy(sc, yts[s])
                scs.append(sc)
            for s in range(2):
                nc.vector.reciprocal(
                    out=r_f[32 * s:32 * s + 1, :], in_=scs[s][64:65, :]
                )
                nc.vector.tensor_copy(
                    r_b[32 * s:32 * s + 1, :], r_f[32 * s:32 * s + 1, :]
                )

            def mk_norm(hp=hp, qi=qi, scs=scs):
                def e():
                    rps = rps_ps.tile([P, 512], F32, tag="rps")
                    nc.tensor.matmul(rps, e8c, r_b, start=True, stop=True)
                    for s in range(2):
                        nc.vector.tensor_mul(
                            out=ytu[64 * s:64 * s + 64, hp,
                                    qi * 512:(qi + 1) * 512],
                            in0=scs[s][0:64, :],
                            in1=rps[64 * s:64 * s + 64, :],
                        )
                return e
            deferred.append(mk_norm())

            if hp == 3:
                for it in range(4 * qi, 4 * qi + 4):
                    for nb in range(2):
                        def mk_proj(it=it, nb=nb):
                            def e():
                                ops = aux_ps.tile([P, 512], F32, tag="aux")
                                for pc in range(4):
                                    nc.tensor.matmul(
                                        ops, ytu[:, pc, it * P:(it + 1) * P],
                                        wpt_sb[:, pc, nb * 512:(nb + 1) * 512],
                                        start=(pc == 0), stop=(pc == 3),
                                    )
                                osb = osb_pool.tile([P, 512], BF16, tag="osb")
                                nc.vector.tensor_copy(osb, ops)
                                nc.sync.dma_start(
                                    out[it * P:(it + 1) * P,
                                        nb * 512:(nb + 1) * 512], osb
                                )
                            return e
                        deferred.append(mk_proj())

    while deferred:
        deferred.popleft()()
    while pending:
        pending.popleft()()


def _prep_inputs(x, w_qkv, w_proj):
    """Build the 8 per-core input maps (host-side sharding + bf16 casts)."""
    xts = [np.ascontiguousarray(x[b].T).astype(BF16_NP) for b in range(B)]
    wqkvts, wpts = [], []
    for hg in range(HG):
        s = hg * CG
        wq = w_qkv[s:s + CG]
        wk = w_qkv[C + s:C + s + CG]
        wv = w_qkv[2 * C + s:2 * C + s + CG]
        wqkvts.append(
            np.ascontiguousarray(np.concatenate([wq, wk, wv], 0).T).astype(BF16_NP)
        )
        wpts.append(np.ascontiguousarray(w_proj[:, s:s + CG].T).astype(BF16_NP))
    in_maps = []
    for c in range(8):
        b, hg = c // 2, c % 2
        in_maps.append({"xt": xts[b], "wqkvt": wqkvts[hg], "wpt": wpts[hg]})
    return in_maps


def kernel(x, w_qkv, w_proj):
    x = np.asarray(x, dtype=np.float32)
    w_qkv = np.asarray(w_qkv, dtype=np.float32)
    w_proj = np.asarray(w_proj, dtype=np.float32)

    if "nc" not in _CACHE:
        _CACHE["nc"] = _build_core_program()
    nc = _CACHE["nc"]

    in_maps = _prep_inputs(x, w_qkv, w_proj)
    res = run_bass_kernel_spmd(nc, in_maps, core_ids=list(range(8)))
    outs = [r["out"] for r in res.results]
    full = np.empty((B, T, C), dtype=np.float32)
    for b in range(B):
        full[b] = outs[2 * b].astype(np.float32) + outs[2 * b + 1].astype(np.float32)
    return full


# revision 7
# speedup vs baseline: 1.7813x; 1.2339x over previous
"""Causal self-attention on 8 TRN2 NeuronCores — bf16 tensor-parallel version.

Problem: x[4, 2048, 1024], w_qkv[3072, 1024], w_proj[1024, 1024],
16 heads x 64 dims, causal softmax attention, output [4, 2048, 1024].

Sharding: core c handles (batch b = c//2, head-group hg = c%2).
Each head-group = 8 heads = 512 channels. Tensor-parallel over heads:
each core computes a *partial* projection output [2048, 1024] in bf16;
the host sums the two head-group partials per batch in fp32.

All matmuls are bf16 (rel-err budget 2e-2; bf16 keeps us ~4e-3):
  Phase A:  QT = Wq @ X^T   [512, 2048]   (head-pair channels on partitions)
            KT = Wk @ X^T   [512, 2048]
            V  = X @ Wv^T   [2048, 512]   (+ ones column per head)
            emitted as 8-matmul groups; all but the first ib=0 slices are
            drained one-per-j inside the attention loops so PE fills the
            ACT-paced softmax era and ACT starts exp'ing early.
  Attention, qi-major: per 512-query block qi, per head-pair hp, per
  128-key tile j:
            ST pair: two K=64 row-group-concurrent matmuls -> one
              [128, 1024] 2-bank PSUM tile (head s at cols 512s..)
            diagonal tiles truncated to valid queries (n0 = 128*(j-4qi));
              the leading 128x128 triangle masked by accumulating
              (-1e5*I) @ tri  (one extra N=128 matmul per head)
            PT pair = exp(0.125 * ST) in ONE activation over both heads
            YT_s += [V_h | 1]^T @ PT_s  accumulated over j  ([65, 512]
              PSUM; row 64 = softmax denominators)
  Normalize (batched per qi): sc = copy(YT) per head (frees PSUM);
            DMA the 8 denominator rows into dq[8, 512]; ONE reciprocal;
            per hp: rps = e8^T @ r broadcasts r over the 64 dims of each
            head; ytu = sc * rps (bf16).  All of this is deferred into
            the next query block's j-loops to keep the PE FIFO moving.
  Proj:     out[it*128.., nb*512..] partial = sum_pc ytu_pc^T-contracted
            with w_proj slice; bf16 out via DMA.  Also deferred.
"""

import numpy as np
from collections import deque
from contextlib import ExitStack

import ml_dtypes

import concourse.bass as bass
import concourse.tile as tile
from concourse import bacc, mybir
from concourse.bass_utils import run_bass_kernel_spmd

B, T, C, H, D = 4, 2048, 1024, 16, 64
HG = 2                 # head groups (tensor-parallel ways)
CG = 512               # channels per head group
P = 128
NQI = T // 512         # 4 query blocks
NJT = T // P           # 16 key tiles
NEG = -1.0e5           # causal mask additive constant (exp(0.125*NEG) == 0)
F32 = mybir.dt.float32
BF16 = mybir.dt.bfloat16
BF16_NP = ml_dtypes.bfloat16

_CACHE = {}


def _build_core_program():
    nc = bacc.Bacc("TRN2", target_bir_lowering=False, debug=False, num_devices=8)
    xt = nc.dram_tensor("xt", [C, T], BF16, kind="ExternalInput").ap()
    wqkvt = nc.dram_tensor("wqkvt", [C, 3 * CG], BF16, kind="ExternalInput").ap()
    wpt = nc.dram_tensor("wpt", [CG, C], BF16, kind="ExternalInput").ap()
    out = nc.dram_tensor("out", [T, C], BF16, kind="ExternalOutput").ap()

    with tile.TileContext(nc) as tc:
        with ExitStack() as ctx:
            _attention(ctx, tc, xt, wqkvt, wpt, out)
    nc.compile()
    return nc


def _attention(ctx, tc, xt, wqkvt, wpt, out):
    nc = tc.nc

    # ---------------- persistent SBUF ----------------
    persist = ctx.enter_context(tc.tile_pool(name="persist", bufs=1))
    qt = persist.tile([P, 4, T], BF16, tag="qt")       # QT[hp*128+p, t] at [p, hp, t]
    kt = persist.tile([P, 4, T], BF16, tag="kt")
    v = persist.tile([P, NJT, 8 * 65], BF16, tag="v")  # [V_h | 1] per key tile
    ytu = persist.tile([P, 4, T], BF16, tag="ytu")     # normalized YT (bf16)
    wpt_sb = persist.tile([P, 4, C], BF16, tag="wpt")

    # ---------------- constants ----------------
    consts = ctx.enter_context(tc.tile_pool(name="consts", bufs=1))
    with ExitStack() as cstage:
        stage = cstage.enter_context(tc.tile_pool(name="cstage", bufs=2))
        ns = stage.tile([P, P], F32, tag="cst", name="negIs")
        nc.gpsimd.memset(ns, 0.0)
        # keep 0 where (q - p) != 0, fill NEG on the diagonal -> NEG * I
        nc.gpsimd.affine_select(
            out=ns, in_=ns, compare_op=mybir.AluOpType.not_equal, fill=NEG,
            base=0, pattern=[[-1, P]], channel_multiplier=1,
        )
        neg_i = consts.tile([P, P], BF16, tag="negI")
        nc.vector.tensor_copy(neg_i, ns)

        ts_ = stage.tile([P, P], F32, tag="cst", name="tris")
        nc.gpsimd.memset(ts_, 0.0)
        # keep 0 where (q - p) >= 0 (valid), fill 1 where q < p (masked)
        nc.gpsimd.affine_select(
            out=ts_, in_=ts_, compare_op=mybir.AluOpType.is_ge, fill=1.0,
            base=0, pattern=[[1, P]], channel_multiplier=-1,
        )
        tri = consts.tile([P, P], BF16, tag="tri")
        nc.vector.tensor_copy(tri, ts_)

        # e8[pc][p, c] = 1 iff p == 2*pc + c//64 : broadcasts r rows onto
        # the 64 channel-partitions of each head of pair pc.
        es = stage.tile([8, 512], F32, tag="cst2", name="e8s")
        nc.gpsimd.memset(es, 0.0)
        e4d = es.rearrange("p (c a b) -> p c a b", a=2, b=64)
        nc.gpsimd.affine_select(
            out=e4d, in_=e4d, compare_op=mybir.AluOpType.not_equal, fill=1.0,
            base=0, pattern=[[-2, 4], [-1, 2], [0, 64]], channel_multiplier=1,
        )
        e8 = []
        for pc in range(4):
            t = consts.tile([8, P], BF16, tag=f"e8_{pc}", name=f"e8_{pc}")
            nc.vector.tensor_copy(t, es[:, pc * P:(pc + 1) * P])
            e8.append(t)

    v4 = v.rearrange("p j (h f) -> p j h f", f=65)
    nc.gpsimd.memset(v4[:, :, :, 64:65], 1.0)   # ones column of each head

    # ---------------- input DMA ----------------
    xw = ctx.enter_context(tc.tile_pool(name="xw", bufs=8))
    xs, ws = [], []
    for cc in range(8):
        xc = xw.tile([P, T], BF16, tag="x", name=f"x{cc}")
        for h in range(2):
            nc.sync.dma_start(
                xc[:, h * 1024:(h + 1) * 1024],
                xt[cc * P:(cc + 1) * P, h * 1024:(h + 1) * 1024],
            )
        wc = xw.tile([P, 3 * CG], BF16, tag="w", name=f"w{cc}")
        for h in range(2):
            nc.sync.dma_start(
                wc[:, h * 768:(h + 1) * 768],
                wqkvt[cc * P:(cc + 1) * P, h * 768:(h + 1) * 768],
            )
        xs.append(xc)
        ws.append(wc)
    for pc in range(4):
        nc.sync.dma_start(wpt_sb[:, pc, :], wpt[pc * P:(pc + 1) * P, :])

    # ---------------- phase A group emitters ----------------
    def kq_group(pool, tag, dst, hp, ib, wbase):
        def emit():
            ps = pool.tile([P, 512], F32, tag=tag)
            for cc in range(8):
                nc.tensor.matmul(
                    ps, ws[cc][:, wbase + hp * P: wbase + hp * P + P],
                    xs[cc][:, ib * 512:(ib + 1) * 512],
                    start=(cc == 0), stop=(cc == 7),
                )
            nc.vector.tensor_copy(dst[:, hp, ib * 512:(ib + 1) * 512], ps)
        return emit

    def v_group(pool, tag, it):
        def emit():
            ps = pool.tile([P, 512], F32, tag=tag)
            for cc in range(8):
                nc.tensor.matmul(
                    ps, xs[cc][:, it * P:(it + 1) * P],
                    ws[cc][:, 2 * CG:3 * CG],
                    start=(cc == 0), stop=(cc == 7),
                )
            psv = ps.rearrange("p (h e) -> p h e", e=64)
            nc.vector.tensor_copy(v4[:, it, :, 0:64], psv)
        return emit

    # ---------------- upfront phase A (own 2-bank PSUM pool) ----------------
    # qi=0 needs KT/QT ib=0 for every head pair, plus V key tiles 0-3.
    with ExitStack() as upctx:
        up_ps = upctx.enter_context(
            tc.tile_pool(name="up_ps", bufs=2, space="PSUM")
        )
        for hp in range(4):
            kq_group(up_ps, "up", kt, hp, 0, CG)()
            kq_group(up_ps, "up", qt, hp, 0, 0)()
        for it in range(4):
            v_group(up_ps, "up", it)()

    # ---------------- attention-era pools (8 PSUM banks exactly) ----------
    st_ps = ctx.enter_context(tc.tile_pool(name="st_ps", bufs=2, space="PSUM"))
    yt_ps = ctx.enter_context(tc.tile_pool(name="yt_ps", bufs=1, space="PSUM"))
    rps_ps = ctx.enter_context(tc.tile_pool(name="rps_ps", bufs=1, space="PSUM"))
    aux_ps = ctx.enter_context(tc.tile_pool(name="aux_ps", bufs=1, space="PSUM"))
    pt_pool = ctx.enter_context(tc.tile_pool(name="pt", bufs=3))
    sc_pool = ctx.enter_context(tc.tile_pool(name="sc", bufs=2))
    dq_pool = ctx.enter_context(tc.tile_pool(name="dq", bufs=2))
    r8_pool = ctx.enter_context(tc.tile_pool(name="r8", bufs=2))
    osb_pool = ctx.enter_context(tc.tile_pool(name="osb", bufs=2))

    # remaining phase A work, drained one group per j inside attention.
    # Order matters: query block qi needs KT/QT ib<=qi and V tiles <=4qi+3.
    pending = deque()
    for ib in range(1, 4):
        for it in range(4 * ib, 4 * ib + 4):
            pending.append(v_group(aux_ps, "aux", it))
        for hp in range(4):
            pending.append(kq_group(aux_ps, "aux", kt, hp, ib, CG))
            pending.append(kq_group(aux_ps, "aux", qt, hp, ib, 0))

    # ---------------- attention ----------------
    deferred = deque()    # prev block's normalize + proj, flushed 1/j
    for qi in range(NQI):
        njt = 4 * qi + 4
        dq = dq_pool.tile([8, 512], F32, tag="dq")
        scs = {}
        for hp in range(4):
            yts = [yt_ps.tile([65, 512], F32, tag=f"yt{s}", name=f"yt{s}")
                   for s in range(2)]
            yt_pend = None
            for j in range(njt):
                o = j - 4 * qi          # >= 0 on diagonal tiles
                n0 = 128 * o if o >= 0 else 0
                stp = st_ps.tile([P, 1024], F32, tag="st")
                for s in range(2):
                    r0 = 64 * s
                    nc.tensor.matmul(
                        stp[:, s * 512 + n0: (s + 1) * 512],
                        kt[r0:r0 + 64, hp, j * P:(j + 1) * P],
                        qt[r0:r0 + 64, hp, qi * 512 + n0: (qi + 1) * 512],
                        start=True, stop=(o < 0),
                    )
                    if o >= 0:
                        nc.tensor.matmul(
                            stp[:, s * 512 + n0: s * 512 + n0 + P],
                            neg_i, tri, start=False, stop=True,
                        )
                ptp = pt_pool.tile([P, 1024], BF16, tag="pt")
                stv = stp.rearrange("p (s q) -> p s q", s=2)[:, :, n0:512]
                ptv = ptp.rearrange("p (s q) -> p s q", s=2)[:, :, n0:512]
                nc.scalar.activation(
                    ptv, stv, mybir.ActivationFunctionType.Exp, scale=0.125
                )
                if deferred:
                    deferred.popleft()()
                if pending:
                    pending.popleft()()
                if yt_pend is not None:
                    yt_pend()

                def mk_yt(j=j, n0=n0, ptp=ptp, hp=hp, yts=yts,
                          last=(j == njt - 1)):
                    def e():
                        for s in range(2):
                            h = 2 * hp + s
                            nc.tensor.matmul(
                                yts[s][:, n0:512],
                                v[:, j, h * 65:(h + 1) * 65],
                                ptp[:, s * 512 + n0:(s + 1) * 512],
                                start=(j == 0), stop=last,
                            )
                    return e
                yt_pend = mk_yt()
            yt_pend()

            # copy YT out of PSUM (frees the banks), stash denominator rows
            for s in range(2):
                sc = sc_pool.tile([65, 512], F32, tag=f"sc{hp}{s}",
                                  name=f"sc{hp}{s}")
                nc.vector.tensor_copy(sc, yts[s])
                nc.sync.dma_start(dq[2 * hp + s:2 * hp + s + 1, :],
                                  sc[64:65, :])
                scs[(hp, s)] = sc

        # ---- deferred normalize (one batched reciprocal per query block)
        rhold = {}

        def mk_recip(dq=dq, rhold=rhold):
            def e():
                rf = r8_pool.tile([8, 512], F32, tag="rf")
                rb = r8_pool.tile([8, 512], BF16, tag="rb")
                nc.vector.reciprocal(out=rf, in_=dq)
                nc.vector.tensor_copy(rb, rf)
                rhold["rb"] = rb
            return e
        deferred.append(mk_recip())

        for hp in range(4):
            def mk_norm(hp=hp, qi=qi, scs=scs, rhold=rhold):
                def e():
                    rps = rps_ps.tile([P, 512], F32, tag="rps")
                    nc.tensor.matmul(rps, e8[hp], rhold["rb"],
                                     start=True, stop=True)
                    for s in range(2):
                        nc.vector.tensor_mul(
                            out=ytu[64 * s:64 * s + 64, hp,
                                    qi * 512:(qi + 1) * 512],
                            in0=scs[(hp, s)][0:64, :],
                            in1=rps[64 * s:64 * s + 64, :],
                        )
                return e
            deferred.append(mk_norm())

        for it in range(4 * qi, 4 * qi + 4):
            for nb in range(2):
                def mk_proj(it=it, nb=nb):
                    def e():
                        ops = aux_ps.tile([P, 512], F32, tag="aux")
                        for pc in range(4):
                            nc.tensor.matmul(
                                ops, ytu[:, pc, it * P:(it + 1) * P],
                                wpt_sb[:, pc, nb * 512:(nb + 1) * 512],
                                start=(pc == 0), stop=(pc == 3),
                            )
                        osb = osb_pool.tile([P, 512], BF16, tag="osb")
                        nc.vector.tensor_copy(osb, ops)
                        nc.sync.dma_start(
                            out[it * P:(it + 1) * P,
                                nb * 512:(nb + 1) * 512], osb
                        )
                    return e
                deferred.append(mk_proj())

    while deferred:
        deferred.popleft()()
    while pending:
        pending.popleft()()


def _prep_inputs(x, w_qkv, w_proj):
    """Build the 8 per-core input maps (host-side sharding + bf16 casts)."""
    xts = [np.ascontiguousarray(x[b].T).astype(BF16_NP) for b in range(B)]
    wqkvts, wpts = [], []
    for hg in range(HG):
        s = hg * CG
        wq = w_qkv[s:s + CG]
        wk = w_qkv[C + s:C + s + CG]
        wv = w_qkv[2 * C + s:2 * C + s + CG]
        wqkvts.append(
            np.ascontiguousarray(np.concatenate([wq, wk, wv], 0).T).astype(BF16_NP)
        )
        wpts.append(np.ascontiguousarray(w_proj[:, s:s + CG].T).astype(BF16_NP))
    in_maps = []
    for c in range(8):
        b, hg = c // 2, c % 2
        in_maps.append({"xt": xts[b], "wqkvt": wqkvts[hg], "wpt": wpts[hg]})
    return in_maps


def kernel(x, w_qkv, w_proj):
    x = np.asarray(x, dtype=np.float32)
    w_qkv = np.asarray(w_qkv, dtype=np.float32)
    w_proj = np.asarray(w_proj, dtype=np.float32)

    if "nc" not in _CACHE:
        _CACHE["nc"] = _build_core_program()
    nc = _CACHE["nc"]

    in_maps = _prep_inputs(x, w_qkv, w_proj)
    res = run_bass_kernel_spmd(nc, in_maps, core_ids=list(range(8)))
    outs = [r["out"] for r in res.results]
    full = np.empty((B, T, C), dtype=np.float32)
    for b in range(B):
        full[b] = outs[2 * b].astype(np.float32) + outs[2 * b + 1].astype(np.float32)
    return full


# revision 13
# speedup vs baseline: 1.8127x; 1.0176x over previous
"""Causal self-attention on 8 TRN2 NeuronCores — bf16 tensor-parallel version.

Problem: x[4, 2048, 1024], w_qkv[3072, 1024], w_proj[1024, 1024],
16 heads x 64 dims, causal softmax attention, output [4, 2048, 1024].

Sharding: core c handles (batch b = c//2, head-group hg = c%2).
Each head-group = 8 heads = 512 channels. Tensor-parallel over heads:
each core computes a *partial* projection output [2048, 1024] in bf16;
the host sums the two head-group partials per batch in fp32.

All matmuls are bf16 (rel-err budget 2e-2; bf16 keeps us ~4e-3):
  Phase A:  QT = Wq @ X^T   [512, 2048]   (head-pair channels on partitions)
            KT = Wk @ X^T   [512, 2048]
            V  = X @ Wv^T   [2048, 512]   (+ ones column per head)
            emitted as 8-matmul groups; all but the first ib=0 slices are
            drained one-per-j inside the attention loops so PE fills the
            ACT-paced softmax era and ACT starts exp'ing early.
  Attention, qi-major: per 512-query block qi, per head-pair hp, per
  128-key tile j:
            ST pair: two K=64 row-group-concurrent matmuls -> one
              [128, 1024] 2-bank PSUM tile (head s at cols 512s..)
            diagonal tiles truncated to valid queries (n0 = 128*(j-4qi));
              the leading 128x128 triangle masked by accumulating
              (-1e5*I) @ tri  (one extra N=128 matmul per head)
            PT pair = exp(0.125 * ST) in ONE activation over both heads
            YT_s += [V_h | 1]^T @ PT_s  accumulated over j  ([65, 512]
              PSUM; row 64 = softmax denominators)
  Normalize (batched per qi): sc = copy(YT) per head (frees PSUM);
            DMA the 8 denominator rows into dq[8, 512]; ONE reciprocal;
            per hp: rps = e8^T @ r broadcasts r over the 64 dims of each
            head; ytu = sc * rps (bf16).  All of this is deferred into
            the next query block's j-loops to keep the PE FIFO moving.
  Proj:     out[it*128.., nb*512..] partial = sum_pc ytu_pc^T-contracted
            with w_proj slice; bf16 out via DMA.  Also deferred.
"""

import numpy as np
from collections import deque
from contextlib import ExitStack

import ml_dtypes

import concourse.bass as bass
import concourse.tile as tile
from concourse import bacc, mybir
from concourse.bass_utils import run_bass_kernel_spmd

B, T, C, H, D = 4, 2048, 1024, 16, 64
HG = 2                 # head groups (tensor-parallel ways)
CG = 512               # channels per head group
P = 128
NQI = T // 512         # 4 query blocks
NJT = T // P           # 16 key tiles
NEG = -1.0e5           # causal mask additive constant (exp(0.125*NEG) == 0)
F32 = mybir.dt.float32
BF16 = mybir.dt.bfloat16
BF16_NP = ml_dtypes.bfloat16

_CACHE = {}


def _build_core_program():
    nc = bacc.Bacc("TRN2", target_bir_lowering=False, debug=False, num_devices=8)
    xt = nc.dram_tensor("xt", [C, T], BF16, kind="ExternalInput").ap()
    wqkvt = nc.dram_tensor("wqkvt", [C, 3 * CG], BF16, kind="ExternalInput").ap()
    wpt = nc.dram_tensor("wpt", [CG, C], BF16, kind="ExternalInput").ap()
    out = nc.dram_tensor("out", [T, C], BF16, kind="ExternalOutput").ap()

    with tile.TileContext(nc) as tc:
        with ExitStack() as ctx:
            _attention(ctx, tc, xt, wqkvt, wpt, out)
    nc.compile()
    return nc


def _attention(ctx, tc, xt, wqkvt, wpt, out):
    nc = tc.nc

    # ---------------- persistent SBUF ----------------
    persist = ctx.enter_context(tc.tile_pool(name="persist", bufs=1))
    qt = persist.tile([P, 4, T], BF16, tag="qt")       # QT[hp*128+p, t] at [p, hp, t]
    kt = persist.tile([P, 4, T], BF16, tag="kt")
    v = persist.tile([P, NJT, 8 * 65], BF16, tag="v")  # [V_h | 1] per key tile
    ytu = persist.tile([P, 4, T], BF16, tag="ytu")     # normalized YT (bf16)
    wpt_sb = persist.tile([P, 4, C], BF16, tag="wpt")

    # ---------------- constants ----------------
    consts = ctx.enter_context(tc.tile_pool(name="consts", bufs=1))
    with ExitStack() as cstage:
        stage = cstage.enter_context(tc.tile_pool(name="cstage", bufs=2))
        ns = stage.tile([P, P], F32, tag="cst", name="negIs")
        nc.gpsimd.memset(ns, 0.0)
        # keep 0 where (q - p) != 0, fill NEG on the diagonal -> NEG * I
        nc.gpsimd.affine_select(
            out=ns, in_=ns, compare_op=mybir.AluOpType.not_equal, fill=NEG,
            base=0, pattern=[[-1, P]], channel_multiplier=1,
        )
        neg_i = consts.tile([P, P], BF16, tag="negI")
        nc.vector.tensor_copy(neg_i, ns)

        ts_ = stage.tile([P, P], F32, tag="cst", name="tris")
        nc.gpsimd.memset(ts_, 0.0)
        # keep 0 where (q - p) >= 0 (valid), fill 1 where q < p (masked)
        nc.gpsimd.affine_select(
            out=ts_, in_=ts_, compare_op=mybir.AluOpType.is_ge, fill=1.0,
            base=0, pattern=[[1, P]], channel_multiplier=-1,
        )
        tri = consts.tile([P, P], BF16, tag="tri")
        nc.vector.tensor_copy(tri, ts_)

        # e8[pc][p, c] = 1 iff p == 2*pc + c//64 : broadcasts r rows onto
        # the 64 channel-partitions of each head of pair pc.
        es = stage.tile([8, 512], F32, tag="cst2", name="e8s")
        nc.gpsimd.memset(es, 0.0)
        e4d = es.rearrange("p (c a b) -> p c a b", a=2, b=64)
        nc.gpsimd.affine_select(
            out=e4d, in_=e4d, compare_op=mybir.AluOpType.not_equal, fill=1.0,
            base=0, pattern=[[-2, 4], [-1, 2], [0, 64]], channel_multiplier=1,
        )
        e8 = []
        for pc in range(4):
            t = consts.tile([8, P], BF16, tag=f"e8_{pc}", name=f"e8_{pc}")
            nc.vector.tensor_copy(t, es[:, pc * P:(pc + 1) * P])
            e8.append(t)
        # e2[s, c] = 1 iff c//64 == s (2-row variant for the hp==3 chunk)
        e2s = stage.tile([2, P], F32, tag="cst3", name="e2s")
        nc.gpsimd.memset(e2s, 0.0)
        e2d = e2s.rearrange("p (a b) -> p a b", b=64)
        nc.gpsimd.affine_select(
            out=e2d, in_=e2d, compare_op=mybir.AluOpType.not_equal, fill=1.0,
            base=0, pattern=[[-1, 2], [0, 64]], channel_multiplier=1,
        )
        e2 = consts.tile([2, P], BF16, tag="e2")
        nc.vector.tensor_copy(e2, e2s)

    v4 = v.rearrange("p j (h f) -> p j h f", f=65)
    nc.gpsimd.memset(v4[:, :, :, 64:65], 1.0)   # ones column of each head

    # ---------------- input DMA ----------------
    xw = ctx.enter_context(tc.tile_pool(name="xw", bufs=8))
    xs, ws = [], []
    for cc in range(8):
        xc = xw.tile([P, T], BF16, tag="x", name=f"x{cc}")
        for h in range(2):
            nc.sync.dma_start(
                xc[:, h * 1024:(h + 1) * 1024],
                xt[cc * P:(cc + 1) * P, h * 1024:(h + 1) * 1024],
            )
        wc = xw.tile([P, 3 * CG], BF16, tag="w", name=f"w{cc}")
        for h in range(2):
            nc.sync.dma_start(
                wc[:, h * 768:(h + 1) * 768],
                wqkvt[cc * P:(cc + 1) * P, h * 768:(h + 1) * 768],
            )
        xs.append(xc)
        ws.append(wc)
    for pc in range(4):
        nc.sync.dma_start(wpt_sb[:, pc, :], wpt[pc * P:(pc + 1) * P, :])

    # ---------------- phase A group emitters ----------------
    def kq_group(pool, tag, dst, hp, ib, wbase):
        def emit():
            ps = pool.tile([P, 512], F32, tag=tag)
            for cc in range(8):
                nc.tensor.matmul(
                    ps, ws[cc][:, wbase + hp * P: wbase + hp * P + P],
                    xs[cc][:, ib * 512:(ib + 1) * 512],
                    start=(cc == 0), stop=(cc == 7),
                )
            nc.vector.tensor_copy(dst[:, hp, ib * 512:(ib + 1) * 512], ps)
        return emit

    def v_group(pool, tag, it):
        def emit():
            ps = pool.tile([P, 512], F32, tag=tag)
            for cc in range(8):
                nc.tensor.matmul(
                    ps, xs[cc][:, it * P:(it + 1) * P],
                    ws[cc][:, 2 * CG:3 * CG],
                    start=(cc == 0), stop=(cc == 7),
                )
            psv = ps.rearrange("p (h e) -> p h e", e=64)
            nc.vector.tensor_copy(v4[:, it, :, 0:64], psv)
        return emit

    # ---------------- upfront phase A (own 2-bank PSUM pool) ----------------
    # only what the very first j-iteration needs; the rest drains as
    # `pending` inside the attention loops.
    with ExitStack() as upctx:
        up_ps = upctx.enter_context(
            tc.tile_pool(name="up_ps", bufs=2, space="PSUM")
        )
        kq_group(up_ps, "up", kt, 0, 0, CG)()
        kq_group(up_ps, "up", qt, 0, 0, 0)()

    # ---------------- attention-era pools (8 PSUM banks exactly) ----------
    st_ps = ctx.enter_context(tc.tile_pool(name="st_ps", bufs=2, space="PSUM"))
    yt_ps = ctx.enter_context(tc.tile_pool(name="yt_ps", bufs=1, space="PSUM"))
    rps_ps = ctx.enter_context(tc.tile_pool(name="rps_ps", bufs=1, space="PSUM"))
    aux_ps = ctx.enter_context(tc.tile_pool(name="aux_ps", bufs=1, space="PSUM"))
    pt_pool = ctx.enter_context(tc.tile_pool(name="pt", bufs=3))
    sc_pool = ctx.enter_context(tc.tile_pool(name="sc", bufs=2))
    dq_pool = ctx.enter_context(tc.tile_pool(name="dq", bufs=2))
    r8_pool = ctx.enter_context(tc.tile_pool(name="r8", bufs=2))
    osb_pool = ctx.enter_context(tc.tile_pool(name="osb", bufs=2))

    # remaining phase A work, drained a few groups per j inside attention.
    # Order matters: query block qi needs KT/QT ib<=qi and V tiles <=4qi+3;
    # head pair hp of qi=0 needs its KT/QT ib=0 before its j-loop starts.
    pending = deque()
    for it in range(4):
        pending.append(v_group(aux_ps, "aux", it))
    for hp in range(1, 4):
        pending.append(kq_group(aux_ps, "aux", kt, hp, 0, CG))
        pending.append(kq_group(aux_ps, "aux", qt, hp, 0, 0))
    for ib in range(1, 4):
        for it in range(4 * ib, 4 * ib + 4):
            pending.append(v_group(aux_ps, "aux", it))
        for hp in range(4):
            pending.append(kq_group(aux_ps, "aux", kt, hp, ib, CG))
            pending.append(kq_group(aux_ps, "aux", qt, hp, ib, 0))

    # ---------------- attention ----------------
    deferred = deque()    # normalize + proj closures, flushed a few per j
    for qi in range(NQI):
        njt = 4 * qi + 4
        dq = dq_pool.tile([6, 512], F32, tag="dq")      # denoms, hp 0-2
        dq2 = dq_pool.tile([2, 512], F32, tag="dq2")    # denoms, hp 3
        scs = {}
        for hp in range(4):
            yts = [yt_ps.tile([65, 512], F32, tag=f"yt{s}", name=f"yt{s}")
                   for s in range(2)]
            yt_pend = None
            for j in range(njt):
                o = j - 4 * qi          # >= 0 on diagonal tiles
                n0 = 128 * o if o >= 0 else 0
                stp = st_ps.tile([P, 1024], F32, tag="st")
                for s in range(2):
                    r0 = 64 * s
                    nc.tensor.matmul(
                        stp[:, s * 512 + n0: (s + 1) * 512],
                        kt[r0:r0 + 64, hp, j * P:(j + 1) * P],
                        qt[r0:r0 + 64, hp, qi * 512 + n0: (qi + 1) * 512],
                        start=True, stop=(o < 0),
                    )
                    if o >= 0:
                        nc.tensor.matmul(
                            stp[:, s * 512 + n0: s * 512 + n0 + P],
                            neg_i, tri, start=False, stop=True,
                        )
                ptp = pt_pool.tile([P, 1024], BF16, tag="pt")
                stv = stp.rearrange("p (s q) -> p s q", s=2)[:, :, n0:512]
                ptv = ptp.rearrange("p (s q) -> p s q", s=2)[:, :, n0:512]
                nc.scalar.activation(
                    ptv, stv, mybir.ActivationFunctionType.Exp, scale=0.125
                )
                if deferred:
                    deferred.popleft()()
                quota = 3 if (qi == 0 and hp == 0) else (2 if qi == 0 else 1)
                for _ in range(quota):
                    if pending:
                        pending.popleft()()
                if yt_pend is not None:
                    yt_pend()

                def mk_yt(j=j, n0=n0, ptp=ptp, hp=hp, yts=yts,
                          last=(j == njt - 1)):
                    def e():
                        for s in range(2):
                            h = 2 * hp + s
                            nc.tensor.matmul(
                                yts[s][:, n0:512],
                                v[:, j, h * 65:(h + 1) * 65],
                                ptp[:, s * 512 + n0:(s + 1) * 512],
                                start=(j == 0), stop=last,
                            )
                    return e
                yt_pend = mk_yt()
            yt_pend()

            # copy YT out of PSUM (frees the banks), stash denominator rows
            for s in range(2):
                sc = sc_pool.tile([65, 512], F32, tag=f"sc{hp}{s}",
                                  name=f"sc{hp}{s}")
                nc.vector.tensor_copy(sc, yts[s])
                if hp < 3:
                    nc.sync.dma_start(dq[2 * hp + s:2 * hp + s + 1, :],
                                      sc[64:65, :])
                else:
                    nc.sync.dma_start(dq2[s:s + 1, :], sc[64:65, :])
                scs[(hp, s)] = sc

            def mk_norm(hp2, rkey, rhold, qi=qi, scs=scs):
                def e():
                    rps = rps_ps.tile([P, 512], F32, tag="rps")
                    lhsT = e8[hp2][0:6, :] if hp2 < 3 else e2
                    nc.tensor.matmul(rps, lhsT, rhold[rkey],
                                     start=True, stop=True)
                    for s in range(2):
                        nc.vector.tensor_mul(
                            out=ytu[64 * s:64 * s + 64, hp2,
                                    qi * 512:(qi + 1) * 512],
                            in0=scs[(hp2, s)][0:64, :],
                            in1=rps[64 * s:64 * s + 64, :],
                        )
                return e

            if hp == 2:
                # hp 0-2 denominators complete: their normalize can overlap
                # hp3's j-loop (flushed via the deferred pops)
                rhold = {}

                def mk_recip6(dq=dq, rhold=rhold):
                    def e():
                        rf = r8_pool.tile([6, 512], F32, tag="rf")
                        rb = r8_pool.tile([6, 512], BF16, tag="rb")
                        nc.vector.reciprocal(out=rf, in_=dq)
                        nc.vector.tensor_copy(rb, rf)
                        rhold["rb"] = rb
                    return e
                deferred.append(mk_recip6())
                for h2 in range(3):
                    deferred.append(mk_norm(h2, "rb", rhold))
            elif hp == 3:
                rhold2 = {}

                def mk_recip2(dq2=dq2, rhold2=rhold2):
                    def e():
                        rf = r8_pool.tile([2, 512], F32, tag="rf2")
                        rb = r8_pool.tile([2, 512], BF16, tag="rb2")
                        nc.vector.reciprocal(out=rf, in_=dq2)
                        nc.vector.tensor_copy(rb, rf)
                        rhold2["rb2"] = rb
                    return e
                deferred.append(mk_recip2())
                deferred.append(mk_norm(3, "rb2", rhold2))

        for gi, (it, nb) in enumerate(
                (it, nb) for it in range(4 * qi, 4 * qi + 4)
                for nb in range(2)):
            def mk_proj(it=it, nb=nb, gi=gi):
                def e():
                    # alternate between the two single-buf PSUM pools so
                    # consecutive proj groups pipeline (MM while evac)
                    if gi % 2 == 0:
                        ops = aux_ps.tile([P, 512], F32, tag="aux")
                    else:
                        ops = rps_ps.tile([P, 512], F32, tag="rps")
                    for pc in range(4):
                        nc.tensor.matmul(
                            ops, ytu[:, pc, it * P:(it + 1) * P],
                            wpt_sb[:, pc, nb * 512:(nb + 1) * 512],
                            start=(pc == 0), stop=(pc == 3),
                        )
                    osb = osb_pool.tile([P, 512], BF16, tag="osb")
                    nc.vector.tensor_copy(osb, ops)
                    nc.sync.dma_start(
                        out[it * P:(it + 1) * P,
                            nb * 512:(nb + 1) * 512], osb
                    )
                return e
            deferred.append(mk_proj())

    while deferred:
        deferred.popleft()()
    while pending:
        pending.popleft()()


def _prep_inputs(x, w_qkv, w_proj):
    """Build the 8 per-core input maps (host-side sharding + bf16 casts)."""
    xts = [np.ascontiguousarray(x[b].T).astype(BF16_NP) for b in range(B)]
    wqkvts, wpts = [], []
    for hg in range(HG):
        s = hg * CG
        wq = w_qkv[s:s + CG]
        wk = w_qkv[C + s:C + s + CG]
        wv = w_qkv[2 * C + s:2 * C + s + CG]
        wqkvts.append(
            np.ascontiguousarray(np.concatenate([wq, wk, wv], 0).T).astype(BF16_NP)
        )
        wpts.append(np.ascontiguousarray(w_proj[:, s:s + CG].T).astype(BF16_NP))
    in_maps = []
    for c in range(8):
        b, hg = c // 2, c % 2
        in_maps.append({"xt": xts[b], "wqkvt": wqkvts[hg], "wpt": wpts[hg]})
    return in_maps


def kernel(x, w_qkv, w_proj):
    x = np.asarray(x, dtype=np.float32)
    w_qkv = np.asarray(w_qkv, dtype=np.float32)
    w_proj = np.asarray(w_proj, dtype=np.float32)

    if "nc" not in _CACHE:
        _CACHE["nc"] = _build_core_program()
    nc = _CACHE["nc"]

    in_maps = _prep_inputs(x, w_qkv, w_proj)
    res = run_bass_kernel_spmd(nc, in_maps, core_ids=list(range(8)))
    outs = [r["out"] for r in res.results]
    full = np.empty((B, T, C), dtype=np.float32)
    for b in range(B):
        full[b] = outs[2 * b].astype(np.float32) + outs[2 * b + 1].astype(np.float32)
    return full
